# revision 1
# baseline (speedup 1.0000x reference)
"""BailingMoE linear attention block on 8 trn2 cores (tensor-parallel over heads).

Layout strategy: everything on device is feature-major ("transposed", [feature, T])
so RoPE/attention work on [D, T] tiles and all big matmuls run fp32r at full PE
rate with N=512 moving dims. Per rank (core) r of 8:
  - owns heads 2r, 2r+1 (256 of 2048 feature columns)
  - computes qkv^T + gate^T from shared hidden^T (one pass), RoPE, the chunked
    linear-attention scan, and the row slice of w_dense -> partial y^T.
  - also emits partial sum-of-squares of o over its features: ssq [1, T].
Host: y = (sum_r yT_r * rsqrt(sum_r ssq_r / 2048 + eps)).T  — the RMSNorm row
scale commutes through the dense matmul, so no cross-rank collective is needed.
"""
import numpy as np
import concourse.bass as bass
import concourse.mybir as mybir
import concourse.tile as tile
from concourse import bacc, bass_utils

T, HID, H, D, BLK = 4096, 2048, 16, 128, 256
EPS = 1e-5
THETA = 600000.0
NUM_LAYERS, LAYER_ID = 32, 0
M = 8                 # cores
HPR = H // M          # heads per rank = 2
CW = HPR * D          # feature columns per rank = 256
NCH = T // BLK        # chunks = 16
NK = HID // 128       # 16 k-tiles
TT = 512              # t-slice for projections
NT = T // TT          # 8
FG = (3 * CW + CW) // 128   # 8 f-groups per rank: q0 q1 k0 k1 v0 v1 g0 g1

f32 = mybir.dt.float32
f32r = mybir.dt.float32r
f16 = mybir.dt.float16

_PROGRAM = None


def _build_program():
    nc = bacc.Bacc(trn_type="TRN2")

    hT = nc.dram_tensor("hT", [HID, T], f32r, kind="ExternalInput")
    w_all = nc.dram_tensor("w_all", [HID, 4 * CW], f32r, kind="ExternalInput")
    wd = nc.dram_tensor("wd", [CW, HID], f32r, kind="ExternalInput")
    cosT = nc.dram_tensor("cosT", [D, T], f32r, kind="ExternalInput")
    sinT = nc.dram_tensor("sinT", [D, T], f32r, kind="ExternalInput")
    maskT = nc.dram_tensor("maskT", [128, HPR * 4 * 512], f32r, kind="ExternalInput")
    qdtab = nc.dram_tensor("qdtab", [128, HPR * 512], f32r, kind="ExternalInput")
    kdcol = nc.dram_tensor("kdcol", [128, HPR * 4], f32, kind="ExternalInput")
    bdcol = nc.dram_tensor("bdcol", [128, HPR], f32, kind="ExternalInput")

    y_nat = nc.dram_tensor("y_nat", [T, HID], f32, kind="ExternalOutput")
    ssq = nc.dram_tensor("ssq", [1, T], f32, kind="ExternalOutput")

    with tile.TileContext(nc) as tc:
        with tc.tile_pool(name="persist", bufs=1) as persist:
            # persistent feature-major tensors
            qT = [persist.tile([128, T], f32r, name=f"qT{h}") for h in range(HPR)]
            kT = [persist.tile([128, T], f32r, name=f"kT{h}") for h in range(HPR)]
            vT = [persist.tile([128, T], f32r, name=f"vT{h}") for h in range(HPR)]
            gateT = persist.tile([128, HPR, T], f16, name="gateT")
            ident = persist.tile([128, 128], f32r, name="ident")
            ones_col = persist.tile([128, 1], f32r, name="ones_col")
            ident_f = persist.tile([128, 128], f32, name="ident_f")
            ones_f = persist.tile([128, 1], f32, name="ones_f")

            from concourse.masks import make_identity

            make_identity(nc, ident_f[:])
            nc.vector.tensor_copy(ident[:], ident_f[:])
            nc.gpsimd.memset(ones_f[:], 1.0)
            nc.vector.tensor_copy(ones_col[:], ones_f[:])

            # ---------------- phase A: projections (two passes) + rope --------
            half = D // 2
            for pas in range(2):
                with (
                    tc.tile_pool(name=f"wpool{pas}", bufs=1) as wpool,
                    tc.tile_pool(name=f"stream{pas}", bufs=12) as stream,
                    tc.tile_pool(name=f"ropes{pas}", bufs=1) as ropes,
                    tc.tile_pool(name=f"psA{pas}", bufs=2, space="PSUM") as psA,
                ):
                    w_sb = wpool.tile([128, NK, 512], f32r, name=f"w_sb{pas}")

                    for t in range(NT):
                        tsl = bass.ds(t * TT, TT)
                        hk = []
                        for k in range(NK):
                            hkt = stream.tile(
                                [128, TT], f32r, tag="hk", name=f"hk{pas}_{t}_{k}"
                            )
                            nc.sync.dma_start(
                                hkt[:], hT[k * 128:(k + 1) * 128, t * TT:(t + 1) * TT]
                            )
                            hk.append(hkt)
                            if t == 0:
                                nc.sync.dma_start(
                                    w_sb[:, k, :],
                                    w_all[k * 128:(k + 1) * 128,
                                          pas * 512:(pas + 1) * 512],
                                )
                        accs = []
                        for fg in range(4):
                            a = psA.tile(
                                [128, TT], f32, tag=f"A{fg}", name=f"A{pas}_{t}_{fg}"
                            )
                            accs.append(a)
                        for k in range(NK):
                            for fg in range(4):
                                nc.tensor.matmul(
                                    accs[fg][:],
                                    w_sb[:, k, fg * 128:(fg + 1) * 128],
                                    hk[k][:],
                                    start=(k == 0),
                                    stop=(k == NK - 1),
                                )
                        if pas == 0:
                            # fg 0..1 -> qT, 2..3 -> kT; then rope q/k
                            for h in range(HPR):
                                nc.scalar.copy(qT[h][:, tsl], accs[h][:])
                                nc.scalar.copy(kT[h][:, tsl], accs[2 + h][:])
                            cs = ropes.tile([128, TT], f32r, tag="cs", name=f"cs{t}")
                            nc.sync.dma_start(cs[:], cosT[:, t * TT:(t + 1) * TT])
                            sn = ropes.tile([128, TT], f32r, tag="sn", name=f"sn{t}")
                            nc.sync.dma_start(sn[:], sinT[:, t * TT:(t + 1) * TT])
                            for h in range(HPR):
                                for mat, eng, en in (
                                    (qT[h], nc.vector, "v"),
                                    (kT[h], nc.vector, "g"),
                                ):
                                    x = mat[:, tsl]
                                    nm = f"r{t}_{h}{en}"
                                    tmp = ropes.tile(
                                        [128, TT], f32r, tag=f"tmp{en}", name=f"tp{nm}"
                                    )
                                    t1 = ropes.tile(
                                        [128, TT], f32r, tag=f"t1{en}", name=f"t1{nm}"
                                    )
                                    eng.tensor_mul(t1[:], x[:], cs[:])
                                    eng.tensor_mul(
                                        tmp[0:half, :], x[half:D, :], sn[half:D, :]
                                    )
                                    eng.tensor_mul(
                                        tmp[half:D, :], x[0:half, :], sn[0:half, :]
                                    )
                                    eng.tensor_add(x[:], t1[:], tmp[:])
                        else:
                            # fg 0..1 -> vT, 2..3 -> gate sigmoid
                            for h in range(HPR):
                                nc.scalar.copy(vT[h][:, tsl], accs[h][:])
                                nc.scalar.activation(
                                    gateT[:, h, t * TT:(t + 1) * TT],
                                    accs[2 + h][:],
                                    mybir.ActivationFunctionType.Sigmoid,
                                )

            # ------- attention scan (BLK=512) with fused dense ---------------
            B2 = 512
            NC2 = T // B2
            with tc.tile_pool(name="attsb", bufs=1) as attsb:
              with (
                tc.tile_pool(name="attscratch", bufs=2) as attscr,
                tc.tile_pool(name="attscr1", bufs=1) as attscr1,
                tc.tile_pool(name="psB", bufs=1, space="PSUM") as psB,
              ):
                ssq_sb = attsb.tile([1, T], f32, name="ssq_sb")
                mk_sb = attsb.tile([128, HPR * 4 * B2], f32r, name="mk_sb")
                nc.sync.dma_start(mk_sb[:], maskT[:, :])
                qd_sb = attsb.tile([128, HPR * B2], f32r, name="qd_sb")
                nc.sync.dma_start(qd_sb[:], qdtab[:, :])
                kd_sb = attsb.tile([128, HPR * 4], f32, name="kd_sb")
                nc.sync.dma_start(kd_sb[:], kdcol[:, :])
                bd_sb = attsb.tile([128, HPR], f32, name="bd_sb")
                nc.sync.dma_start(bd_sb[:], bdcol[:, :])
                wd_sb = attsb.tile([128, HPR, HID], f32r, name="wd_sb")
                for kf in range(HPR):
                    nc.sync.dma_start(wd_sb[:, kf, :], wd[kf * 128:(kf + 1) * 128, :])
                S = [attsb.tile([128, 128], f32r, name=f"S{h}") for h in range(HPR)]
                zero_f = attsb.tile([128, 128], f32, name="zero_f")
                nc.vector.memset(zero_f[:], 0.0)
                for h in range(HPR):
                    nc.vector.tensor_copy(S[h][:], zero_f[:])

                for c in range(NC2):
                    csl = bass.ds(c * B2, B2)
                    sq_ps = psB.tile([1, B2], f32, tag="sqps", name=f"sqps{c}")
                    xc = []
                    for h in range(HPR):
                        nm = f"{c}_{h}"
                        qs = qT[h][:, csl]
                        # scores^T per j-subtile, masked
                        m1 = []
                        for jh in range(4):
                            pt = psB.tile(
                                [128, B2], f32, tag=f"pt{jh % 2}", name=f"pt{nm}_{jh}"
                            )
                            nc.tensor.matmul(
                                pt[:],
                                kT[h][:, c * B2 + jh * 128: c * B2 + (jh + 1) * 128],
                                qs,
                                start=True,
                                stop=True,
                            )
                            mt = attscr1.tile(
                                [128, B2], f32r, tag=f"m1_{jh}", name=f"m1{nm}_{jh}"
                            )
                            eng = nc.vector if jh % 2 == 0 else nc.vector
                            eng.tensor_mul(
                                mt[:],
                                pt[:],
                                mk_sb[:, (h * 4 + jh) * B2:(h * 4 + jh + 1) * B2],
                            )
                            m1.append(mt)
                        # natural-layout v and k' via PE transpose
                        vn = []
                        kn = []
                        for jh in range(4):
                            tv = psB.tile([128, 128], f32r, tag="tp", name=f"tv{nm}_{jh}")
                            nc.tensor.transpose(
                                tv[:],
                                vT[h][:, c * B2 + jh * 128: c * B2 + (jh + 1) * 128],
                                ident[:],
                            )
                            vns = attscr.tile(
                                [128, 128], f32r, tag=f"vn{jh}", name=f"vn{nm}_{jh}"
                            )
                            nc.scalar.copy(vns[:], tv[:])
                            vn.append(vns)
                            tk = psB.tile([128, 128], f32r, tag="tp", name=f"tk{nm}_{jh}")
                            nc.tensor.transpose(
                                tk[:],
                                kT[h][:, c * B2 + jh * 128: c * B2 + (jh + 1) * 128],
                                ident[:],
                            )
                            kns = attscr.tile(
                                [128, 128], f32r, tag=f"kn{jh}", name=f"kn{nm}_{jh}"
                            )
                            nc.scalar.activation(
                                kns[:],
                                tk[:],
                                mybir.ActivationFunctionType.Copy,
                                scale=kd_sb[:, h * 4 + jh: h * 4 + jh + 1],
                            )
                            kn.append(kns)
                        qp = attscr.tile([128, B2], f32r, tag="qp", name=f"qp{nm}")
                        nc.gpsimd.tensor_mul(qp[:], qs, qd_sb[:, h * B2:(h + 1) * B2])
                        ops = psB.tile([128, B2], f32, tag="o", name=f"o{nm}")
                        for jh in range(4):
                            nc.tensor.matmul(
                                ops[:], vn[jh][:], m1[jh][:],
                                start=(jh == 0), stop=False,
                            )
                        nc.tensor.matmul(ops[:], S[h][:], qp[:], start=False, stop=True)
                        # ssq partial
                        sq = attscr.tile([128, B2], f32r, tag="sq", name=f"sq{nm}")
                        nc.scalar.square(sq[:], ops[:])
                        nc.tensor.matmul(
                            sq_ps[:], ones_col[:], sq[:],
                            start=(h == 0), stop=(h == HPR - 1),
                        )
                        # x = o * gate  (feature-major slice of this chunk)
                        xch = attscr.tile([128, B2], f32r, tag=f"xc{h}", name=f"xc{nm}")
                        nc.vector.tensor_mul(
                            xch[:], ops[:], gateT[:, h, c * B2:(c + 1) * B2]
                        )
                        xc.append(xch)
                        # state update
                        sps = psB.tile([128, 128], f32, tag="sps", name=f"sps{nm}")
                        for jh in range(4):
                            nc.tensor.matmul(
                                sps[:], kn[jh][:], vn[jh][:],
                                start=(jh == 0), stop=(jh == 3),
                            )
                        nc.vector.scalar_tensor_tensor(
                            out=S[h][:],
                            in0=S[h][:],
                            scalar=bd_sb[:, h: h + 1],
                            in1=sps[:],
                            op0=mybir.AluOpType.mult,
                            op1=mybir.AluOpType.add,
                        )
                    nc.scalar.copy(ssq_sb[:, csl], sq_ps[:])
                    # fused dense for this chunk: y[t, m] natural layout
                    for tb in range(4):
                        for ms in range(4):
                            dps = psB.tile(
                                [128, TT], f32, tag=f"d{ms % 2}", name=f"d{c}_{tb}_{ms}"
                            )
                            for kf in range(HPR):
                                nc.tensor.matmul(
                                    dps[:],
                                    xc[kf][:, tb * 128:(tb + 1) * 128],
                                    wd_sb[:, kf, ms * TT:(ms + 1) * TT],
                                    start=(kf == 0),
                                    stop=(kf == HPR - 1),
                                )
                            dsb = attscr.tile(
                                [128, TT], f32, tag=f"dsb{ms % 2}",
                                name=f"dsb{c}_{tb}_{ms}",
                            )
                            eng = (nc.scalar, nc.vector)[(tb * 4 + ms) % 2]
                            if eng is nc.scalar:
                                eng.copy(dsb[:], dps[:])
                            else:
                                eng.tensor_copy(dsb[:], dps[:])
                            nc.sync.dma_start(
                                y_nat[
                                    (c * 4 + tb) * 128:(c * 4 + tb + 1) * 128,
                                    ms * TT:(ms + 1) * TT,
                                ],
                                dsb[:],
                            )
                nc.sync.dma_start(ssq[:, :], ssq_sb[:])

    nc.compile()
    return nc


def _slopes(n):
    start = 2.0 ** (-(2.0 ** -(np.log2(n) - 3)))
    return np.array([start ** (i + 1) for i in range(n)], dtype=np.float64)


def kernel(hidden_states, positions, w_qkv, w_g, w_dense, g_norm_weight):
    global _PROGRAM
    if _PROGRAM is None:
        _PROGRAM = _build_program()
    nc = _PROGRAM

    hidden_states = np.asarray(hidden_states, dtype=np.float32)
    positions = np.asarray(positions)
    w_qkv = np.asarray(w_qkv, dtype=np.float32)
    w_g = np.asarray(w_g, dtype=np.float32)
    w_dense = np.asarray(w_dense, dtype=np.float32)
    g_norm_weight = np.asarray(g_norm_weight, dtype=np.float32)

    hT = np.ascontiguousarray(hidden_states.T)

    # rope tables, feature-major; sinT carries the rotate-half signs
    half = D // 2
    inv_freq = 1.0 / (THETA ** (np.arange(0, D, 2, dtype=np.float64) / D))
    freqs = positions.astype(np.float64)[:, None] * inv_freq          # [T, 64]
    cos = np.cos(freqs).T                                             # [64, T]
    sin = np.sin(freqs).T
    cosT = np.concatenate([cos, cos], axis=0).astype(np.float32)      # [128, T]
    sinT = np.concatenate([sin, -sin], axis=0).astype(np.float32)

    B2 = 512
    s = _slopes(H) * (1.0 - LAYER_ID / (NUM_LAYERS - 1) + 1e-5)       # [16]
    idx = np.arange(B2, dtype=np.float64)
    diff = idx[:, None] - idx[None, :]
    scale = D ** -0.5
    decay = np.where(
        diff[None, :, :] >= 0, np.exp(-s[:, None, None] * diff[None, :, :]), 0.0
    )                                                                  # [16, B2, B2]
    qd = np.exp(-s[:, None] * (idx[None, :] + 1.0)) * scale            # [16, B2]
    kd = np.exp(-s[:, None] * (B2 - 1.0 - idx[None, :]))               # [16, B2]
    bd = np.exp(-s * B2)                                               # [16]

    in_maps = []
    for r in range(M):
        heads = [M // M * 0 + HPR * r + i for i in range(HPR)]
        cols = slice(r * CW, (r + 1) * CW)
        wq = w_qkv[:, r * CW:(r + 1) * CW]
        wk = w_qkv[:, HID + r * CW: HID + (r + 1) * CW]
        wv = w_qkv[:, 2 * HID + r * CW: 2 * HID + (r + 1) * CW]
        wg = w_g[:, cols]
        w_all = np.concatenate([wq, wk, wv, wg], axis=1)               # [HID, 4*CW]
        wd = (g_norm_weight[cols, None] * w_dense[cols, :]).astype(np.float32)

        mk = np.empty((128, HPR * 4 * B2), np.float32)
        qdt = np.empty((128, HPR * B2), np.float32)
        kdc = np.empty((128, HPR * 4), np.float32)
        bdc = np.empty((128, HPR), np.float32)
        for i, h in enumerate(heads):
            mTh = (decay[h].T * scale).astype(np.float32)              # [j, i]
            for jh in range(4):
                mk[:, (i * 4 + jh) * B2:(i * 4 + jh + 1) * B2] = (
                    mTh[jh * 128:(jh + 1) * 128, :]
                )
                kdc[:, i * 4 + jh] = kd[h, jh * 128:(jh + 1) * 128]
            qdt[:, i * B2:(i + 1) * B2] = np.broadcast_to(
                qd[h][None, :], (128, B2)
            )
            bdc[:, i] = bd[h]

        in_maps.append(
            {
                "hT": hT,
                "w_all": np.ascontiguousarray(w_all),
                "wd": wd,
                "cosT": cosT,
                "sinT": sinT,
                "maskT": mk,
                "qdtab": qdt,
                "kdcol": kdc,
                "bdcol": bdc,
            }
        )

    global _LAST_IN_MAPS
    _LAST_IN_MAPS = in_maps
    results = bass_utils.run_bass_kernel_spmd(nc, in_maps, core_ids=list(range(M)))

    y_sum = np.zeros((T, HID), np.float64)
    ssq_tot = np.zeros((T,), np.float64)
    for r in range(M):
        y_sum += results.results[r]["y_nat"].astype(np.float64)
        ssq_tot += results.results[r]["ssq"][0].astype(np.float64)
    var = ssq_tot / (H * D)
    F = 1.0 / np.sqrt(var + EPS)
    y = y_sum * F[:, None]
    return y.astype(np.float32)



# revision 10
# speedup vs baseline: 1.2717x; 1.2717x over previous
"""BailingMoE linear attention block on 8 trn2 cores (tensor-parallel over heads).

v2: bf16 datapath + single-pass projections + software-pipelined attention.

Per rank r of 8 (heads 2r, 2r+1; 256 of 2048 feature columns):
  - single pass over hidden^T (bf16): feature-major q,k,g projections and
    natural-layout v projection from the same SBUF-resident h tiles.
  - RoPE on q (DVE) and k (Pool) from PSUM-drained f32 scratch -> bf16.
  - chunked linear attention (B=512): triangular-trimmed scores/outputs,
    PE transposes of k for the state update, bf16 small matmuls (1 cyc/row),
    per-head decay state S [128,128] bf16 in SBUF.
  - fused dense (row slice of w_dense, g_norm pre-folded), y partial in bf16.
  - partial sum-of-squares of o emitted as ssq [1, T] f32.
Host: y = (sum_r y_r) * rsqrt(sum_r ssq_r / 2048 + eps) -- the RMSNorm scale
commutes through the dense matmul, so no on-device collective is needed.
"""
import numpy as np
import ml_dtypes
import concourse.bass as bass
import concourse.mybir as mybir
import concourse.tile as tile
from concourse import bacc, bass_utils

T, HID, H, D = 4096, 2048, 16, 128
EPS = 1e-5
THETA = 600000.0
NUM_LAYERS, LAYER_ID = 32, 0
M = 8                 # cores
HPR = H // M          # heads per rank = 2
CW = HPR * D          # feature columns per rank = 256
NK = HID // 128       # 16 k-tiles
TT = 512              # t-slice for projections
NT = T // TT          # 8
B2 = 512              # attention chunk
NCH = T // B2         # 8

f32 = mybir.dt.float32
bf16 = mybir.dt.bfloat16
NPBF16 = ml_dtypes.bfloat16

_PROGRAM = None


def _build_program():
    nc = bacc.Bacc(trn_type="TRN2")

    hTb = nc.dram_tensor("hTb", [128, NK, T], bf16, kind="ExternalInput")
    w_all = nc.dram_tensor("w_all", [128, NK, 1024], bf16, kind="ExternalInput")
    wdd = nc.dram_tensor("wdd", [128, HPR, HID], bf16, kind="ExternalInput")
    cosT = nc.dram_tensor("cosT", [D, T], f32, kind="ExternalInput")
    sinT = nc.dram_tensor("sinT", [D, T], f32, kind="ExternalInput")
    maskT = nc.dram_tensor("maskT", [128, HPR * 4 * B2], f32, kind="ExternalInput")
    qdtab = nc.dram_tensor("qdtab", [128, HPR * B2], bf16, kind="ExternalInput")
    kdcol = nc.dram_tensor("kdcol", [128, HPR * 4], f32, kind="ExternalInput")
    bdcol = nc.dram_tensor("bdcol", [128, HPR], f32, kind="ExternalInput")

    y_nat = nc.dram_tensor("y_nat", [T, HID], bf16, kind="ExternalOutput")
    ssq = nc.dram_tensor("ssq", [1, T], f32, kind="ExternalOutput")

    ACT = mybir.ActivationFunctionType
    ALU = mybir.AluOpType
    half = D // 2

    with tile.TileContext(nc) as tc:
        with tc.tile_pool(name="persist", bufs=1) as persist:
            qTb = persist.tile([128, HPR, T], bf16, name="qTb")
            kTb = persist.tile([128, HPR, T], bf16, name="kTb")
            vN = persist.tile([128, NCH * 4, CW], bf16, name="vN")
            gTb = persist.tile([128, HPR, T], bf16, name="gTb")
            S = [persist.tile([128, 128], bf16, name=f"S{h}") for h in range(HPR)]
            mk_sb = persist.tile([128, HPR * 4 * B2], f32, name="mk_sb")
            qd_sb = persist.tile([128, HPR * B2], bf16, name="qd_sb")
            kd_sb = persist.tile([128, HPR * 4], f32, name="kd_sb")
            bd_sb = persist.tile([128, HPR], f32, name="bd_sb")
            wd_sb = persist.tile([128, HPR, HID], bf16, name="wd_sb")
            ident_f = persist.tile([128, 128], f32, name="ident_f")
            identb = persist.tile([128, 128], bf16, name="identb")
            onesb = persist.tile([128, 1], bf16, name="onesb")
            ssq_sb = persist.tile([1, T], f32, name="ssq_sb")

            from concourse.masks import make_identity

            make_identity(nc, ident_f[:])
            nc.vector.tensor_copy(identb[:], ident_f[:])
            nc.gpsimd.memset(onesb[:], 1.0)
            for h in range(HPR):
                nc.vector.memset(S[h][:], 0.0)
            nc.sync.dma_start(mk_sb[:], maskT[:, :])
            nc.sync.dma_start(qd_sb[:], qdtab[:, :])
            nc.sync.dma_start(kd_sb[:], kdcol[:, :])
            nc.sync.dma_start(bd_sb[:], bdcol[:, :])
            nc.sync.dma_start(wd_sb[:], wdd[:, :, :])

            # ---------------- phase A: projections (single h pass) + rope ----
            with (
                tc.tile_pool(name="wpool", bufs=1) as wpool,
                tc.tile_pool(name="astream", bufs=2) as astream,
                tc.tile_pool(name="ascr", bufs=1) as ascr,
                tc.tile_pool(name="psA", bufs=1, space="PSUM") as psA,
            ):
                w_sb = wpool.tile([128, NK, 1024], bf16, name="w_sb")

                for t in range(NT):
                    tsl = bass.ds(t * TT, TT)
                    hk = astream.tile([128, NK, TT], bf16, tag="hk", name=f"hk{t}")
                    if t == 0:
                        # interleave h and w loads so the first matmuls gate
                        # on small transfers
                        for k2 in range(0, NK, 2):
                            nc.sync.dma_start(
                                hk[:, k2:k2 + 2, :],
                                hTb[:, k2:k2 + 2, t * TT:(t + 1) * TT],
                            )
                            if k2 % 4 == 0 and k2 // 4 * 4 < NK:
                                k4 = k2 // 4 * 4
                                nc.sync.dma_start(
                                    w_sb[:, k4:k4 + 4, :], w_all[:, k4:k4 + 4, :]
                                )
                    else:
                        for k4 in range(0, NK, 4):
                            nc.sync.dma_start(
                                hk[:, k4:k4 + 4, :],
                                hTb[:, k4:k4 + 4, t * TT:(t + 1) * TT],
                            )
                    cs = astream.tile([128, TT], f32, tag="cs", name=f"cs{t}")
                    nc.sync.dma_start(cs[:], cosT[:, t * TT:(t + 1) * TT])
                    sn = astream.tile([128, TT], f32, tag="sn", name=f"sn{t}")
                    nc.sync.dma_start(sn[:], sinT[:, t * TT:(t + 1) * TT])

                    # feature-major: q0 q1 k0 k1 g0 g1
                    for fg in range(6):
                        acc = psA.tile([128, TT], f32, tag=f"A{fg}", name=f"A{t}_{fg}")
                        for k in range(NK):
                            nc.tensor.matmul(
                                acc[:],
                                w_sb[:, k, fg * 128:(fg + 1) * 128],
                                hk[:, k, :],
                                start=(k == 0),
                                stop=(k == NK - 1),
                            )
                        if fg < 4:
                            h = fg % 2
                            scr = ascr.tile(
                                [128, TT], f32, tag=f"scr{fg}", name=f"scr{t}_{fg}"
                            )
                            nc.scalar.copy(scr[:], acc[:])
                            eng = nc.vector if fg < 2 else nc.gpsimd
                            en = "v" if fg < 2 else "p"
                            dst = qTb if fg < 2 else kTb
                            t1 = ascr.tile(
                                [128, TT], f32, tag=f"t1{en}", name=f"t1{t}_{fg}"
                            )
                            tmp = ascr.tile(
                                [128, TT], f32, tag=f"tm{en}", name=f"tm{t}_{fg}"
                            )
                            eng.tensor_mul(t1[:], scr[:], cs[:])
                            eng.tensor_mul(
                                tmp[0:half, :], scr[half:D, :], sn[half:D, :]
                            )
                            eng.tensor_mul(
                                tmp[half:D, :], scr[0:half, :], sn[0:half, :]
                            )
                            eng.tensor_add(dst[:, h, tsl], t1[:], tmp[:])
                        else:
                            nc.scalar.activation(
                                gTb[:, fg - 4, tsl], acc[:], ACT.Sigmoid
                            )

                    # natural-layout v (both heads' 256 features at once)
                    vb0 = psA.tile([128, 512], f32, tag="VB0", name=f"VB0_{t}")
                    vb1 = psA.tile([128, 512], f32, tag="VB1", name=f"VB1_{t}")
                    accv = [
                        vb0[:, 0:256], vb0[:, 256:512],
                        vb1[:, 0:256], vb1[:, 256:512],
                    ]
                    for tb in range(4):
                        for k in range(NK):
                            nc.tensor.matmul(
                                accv[tb],
                                hk[:, k, tb * 128:(tb + 1) * 128],
                                w_sb[:, k, 768:1024],
                                start=(k == 0),
                                stop=(k == NK - 1),
                            )
                        nc.scalar.copy(vN[:, t * 4 + tb, :], accv[tb])

            # -------- phase B: attention scan + fused dense, sw-pipelined ----
            with (
                tc.tile_pool(name="bscr", bufs=1) as bscr,
                tc.tile_pool(name="bscr2", bufs=2) as bscr2,
                tc.tile_pool(name="psB", bufs=1, space="PSUM") as psB,
            ):
                prev_xc = None   # (chunk idx, [xc_h0, xc_h1]) from prev chunk

                def scores_stage(c, h):
                    """PE: scores (triangular) + k transposes. Returns psum APs."""
                    nm = f"{c}_{h}"
                    qs = qTb[:, h, c * B2:(c + 1) * B2]
                    pta = psB.tile([128, 512], f32, tag="PT0", name=f"pta{nm}")
                    ptb = psB.tile([128, 512], f32, tag="PT1", name=f"ptb{nm}")
                    ptc = psB.tile([128, 512], f32, tag="PT2", name=f"ptc{nm}")
                    tkd = psB.tile([128, 512], bf16, tag="TKB", name=f"tkd{nm}")
                    pts = [
                        pta[:, 0:512], ptb[:, 0:384],
                        ptc[:, 0:256], ptb[:, 384:512],
                    ]
                    tkl = [tkd[:, jh * 128:(jh + 1) * 128] for jh in range(4)]
                    for jh in range(4):
                        nc.tensor.matmul(
                            pts[jh],
                            kTb[:, h, c * B2 + jh * 128: c * B2 + (jh + 1) * 128],
                            qs[:, jh * 128:B2],
                            start=True,
                            stop=True,
                        )
                    for jh in range(4):
                        nc.tensor.transpose(
                            tkl[jh],
                            kTb[:, h, c * B2 + jh * 128: c * B2 + (jh + 1) * 128],
                            identb[:],
                        )
                    return pts, tkl

                def mask_stage(c, h, pts):
                    """DVE: masked scores -> bf16 (trimmed)."""
                    nm = f"{c}_{h}"
                    m1 = []
                    for jh in range(4):
                        w = B2 - jh * 128
                        m1t = bscr.tile(
                            [128, 512], bf16, tag=f"M1_{jh}", name=f"m1{nm}_{jh}"
                        )
                        nc.vector.tensor_mul(
                            m1t[:, 0:w],
                            pts[jh],
                            mk_sb[:, (h * 4 + jh) * B2 + jh * 128:
                                  (h * 4 + jh + 1) * B2],
                        )
                        m1.append(m1t)
                    return m1

                def kns_stage(c, h, tkl):
                    """Act: scaled natural-layout k -> bf16."""
                    nm = f"{c}_{h}"
                    kns = []
                    for jh in range(4):
                        knt = bscr.tile(
                            [128, 128], bf16, tag=f"KN_{jh}", name=f"kn{nm}_{jh}"
                        )
                        nc.scalar.activation(
                            knt[:], tkl[jh], ACT.Copy,
                            scale=kd_sb[:, h * 4 + jh: h * 4 + jh + 1],
                        )
                        kns.append(knt)
                    return kns

                def qp_stage(c, h):
                    nm = f"{c}_{h}"
                    qp = bscr2.tile([128, B2], bf16, tag=f"QP{h}", name=f"qp{nm}")
                    nc.gpsimd.tensor_mul(
                        qp[:], qTb[:, h, c * B2:(c + 1) * B2],
                        qd_sb[:, h * B2:(h + 1) * B2],
                    )
                    return qp

                def dense_half(cp, xc, tbs):
                    """Dense for chunk cp over token subtiles tbs."""
                    for tb in tbs:
                        dsb = bscr2.tile(
                            [128, HID], bf16, tag=f"DSB{tb % 2}",
                            name=f"dsb{cp}_{tb}",
                        )
                        for ms in range(4):
                            dps = psB.tile(
                                [128, 512], f32, tag=f"D{ms % 2}",
                                name=f"d{cp}_{tb}_{ms}",
                            )
                            for kf in range(HPR):
                                nc.tensor.matmul(
                                    dps[:],
                                    xc[kf][:, tb * 128:(tb + 1) * 128],
                                    wd_sb[:, kf, ms * 512:(ms + 1) * 512],
                                    start=(kf == 0),
                                    stop=(kf == HPR - 1),
                                )
                            # PSUM drain: Act takes 10 of 16, DVE 6 of 16
                            idx = tb * 4 + ms
                            if idx in (0, 3, 6, 9, 12, 15):
                                nc.vector.tensor_copy(
                                    dsb[:, ms * 512:(ms + 1) * 512], dps[:]
                                )
                            else:
                                nc.scalar.copy(
                                    dsb[:, ms * 512:(ms + 1) * 512], dps[:]
                                )
                        nc.sync.dma_start(
                            y_nat[(cp * 4 + tb) * 128:(cp * 4 + tb + 1) * 128, :],
                            dsb[:],
                        )

                def out_stage(c, h, m1, kns, qp, sqps, obq):
                    """PE o + sps; Act sq; Pool xch; DVE S update."""
                    nm = f"{c}_{h}"
                    ob = psB.tile([128, 512], f32, tag="OB", name=f"ob{nm}")
                    for ih in range(4):
                        osl = ob[:, ih * 128:(ih + 1) * 128]
                        nc.tensor.matmul(
                            osl, S[h][:], qp[:, ih * 128:(ih + 1) * 128],
                            start=True, stop=False,
                        )
                        for jh in range(ih + 1):
                            nc.tensor.matmul(
                                osl,
                                vN[:, c * 4 + jh, h * 128:(h + 1) * 128],
                                m1[jh][:, (ih - jh) * 128:(ih - jh + 1) * 128],
                                start=False,
                                stop=(jh == ih),
                            )
                    spst = psB.tile([128, 512], f32, tag="SPS", name=f"sps{nm}")
                    sps_ps = spst[:, 0:128]
                    for jh in range(4):
                        nc.tensor.matmul(
                            sps_ps, kns[jh][:],
                            vN[:, c * 4 + jh, h * 128:(h + 1) * 128],
                            start=(jh == 0), stop=(jh == 3),
                        )
                    sq = bscr2.tile([128, B2], bf16, tag="SQ", name=f"sq{nm}")
                    nc.scalar.square(sq[:], ob[:])
                    xch = bscr2.tile([128, B2], bf16, tag=f"XC{h}", name=f"xc{nm}")
                    nc.vector.tensor_mul(xch[:], ob[:], gTb[:, h, c * B2:(c + 1) * B2])
                    nc.vector.scalar_tensor_tensor(
                        out=S[h][:],
                        in0=S[h][:],
                        scalar=bd_sb[:, h: h + 1],
                        in1=sps_ps,
                        op0=ALU.mult,
                        op1=ALU.add,
                    )
                    obq.append(sq)
                    return xch

                for c in range(NCH):
                    obq = []
                    pts0, tkl0 = scores_stage(c, 0)
                    m10 = mask_stage(c, 0, pts0)
                    kns0 = kns_stage(c, 0, tkl0)
                    qp0 = qp_stage(c, 0)
                    if prev_xc is not None:
                        dense_half(prev_xc[0], prev_xc[1], [0, 1])
                    pts1, tkl1 = scores_stage(c, 1)
                    m11 = mask_stage(c, 1, pts1)
                    kns1 = kns_stage(c, 1, tkl1)
                    qp1 = qp_stage(c, 1)
                    if prev_xc is not None:
                        dense_half(prev_xc[0], prev_xc[1], [2, 3])
                    xc0 = out_stage(c, 0, m10, kns0, qp0, None, obq)
                    xc1 = out_stage(c, 1, m11, kns1, qp1, None, obq)
                    # ssq: accumulate both heads via ones-matmul (PT0 bank reuse)
                    sqps = psB.tile([1, 512], f32, tag="PT0", name=f"sqps{c}")
                    for h in range(HPR):
                        nc.tensor.matmul(
                            sqps[:], onesb[:], obq[h][:],
                            start=(h == 0), stop=(h == HPR - 1),
                        )
                    nc.scalar.copy(ssq_sb[:, c * B2:(c + 1) * B2], sqps[:])
                    prev_xc = (c, [xc0, xc1])

                dense_half(prev_xc[0], prev_xc[1], [0, 1])
                dense_half(prev_xc[0], prev_xc[1], [2, 3])
                nc.sync.dma_start(ssq[:, :], ssq_sb[:])

    nc.compile()
    return nc


def _slopes(n):
    start = 2.0 ** (-(2.0 ** -(np.log2(n) - 3)))
    return np.array([start ** (i + 1) for i in range(n)], dtype=np.float64)


def kernel(hidden_states, positions, w_qkv, w_g, w_dense, g_norm_weight):
    global _PROGRAM
    if _PROGRAM is None:
        _PROGRAM = _build_program()
    nc = _PROGRAM

    hidden_states = np.asarray(hidden_states, dtype=np.float32)
    positions = np.asarray(positions)
    w_qkv = np.asarray(w_qkv, dtype=np.float32)
    w_g = np.asarray(w_g, dtype=np.float32)
    w_dense = np.asarray(w_dense, dtype=np.float32)
    g_norm_weight = np.asarray(g_norm_weight, dtype=np.float32)

    hT = hidden_states.T.reshape(NK, 128, T).transpose(1, 0, 2)
    hTb = np.ascontiguousarray(hT).astype(NPBF16)

    # rope tables, feature-major; sinT carries the rotate-half signs
    half = D // 2
    inv_freq = 1.0 / (THETA ** (np.arange(0, D, 2, dtype=np.float64) / D))
    freqs = positions.astype(np.float64)[:, None] * inv_freq          # [T, 64]
    cos = np.cos(freqs).T                                             # [64, T]
    sin = np.sin(freqs).T
    cosT = np.concatenate([cos, cos], axis=0).astype(np.float32)      # [128, T]
    sinT = np.concatenate([sin, -sin], axis=0).astype(np.float32)

    s = _slopes(H) * (1.0 - LAYER_ID / (NUM_LAYERS - 1) + 1e-5)       # [16]
    idx = np.arange(B2, dtype=np.float64)
    diff = idx[:, None] - idx[None, :]
    scale = D ** -0.5
    decay = np.where(
        diff[None, :, :] >= 0, np.exp(-s[:, None, None] * diff[None, :, :]), 0.0
    )                                                                  # [16, B2, B2]
    qd = np.exp(-s[:, None] * (idx[None, :] + 1.0)) * scale            # [16, B2]
    kd = np.exp(-s[:, None] * (B2 - 1.0 - idx[None, :]))               # [16, B2]
    bd = np.exp(-s * B2)                                               # [16]

    in_maps = []
    for r in range(M):
        heads = [HPR * r + i for i in range(HPR)]
        cols = slice(r * CW, (r + 1) * CW)
        wq = w_qkv[:, r * CW:(r + 1) * CW]
        wk = w_qkv[:, HID + r * CW: HID + (r + 1) * CW]
        wv = w_qkv[:, 2 * HID + r * CW: 2 * HID + (r + 1) * CW]
        wg = w_g[:, cols]
        wcat = np.concatenate([wq, wk, wg, wv], axis=1)               # [HID, 1024]
        w_allr = np.ascontiguousarray(
            wcat.reshape(NK, 128, 1024).transpose(1, 0, 2)
        ).astype(NPBF16)
        wdr = (g_norm_weight[cols, None] * w_dense[cols, :])
        wdr = np.ascontiguousarray(
            wdr.reshape(HPR, 128, HID).transpose(1, 0, 2)
        ).astype(NPBF16)

        mk = np.empty((128, HPR * 4 * B2), np.float32)
        qdt = np.empty((128, HPR * B2), np.float64)
        kdc = np.empty((128, HPR * 4), np.float32)
        bdc = np.empty((128, HPR), np.float32)
        for i, h in enumerate(heads):
            mTh = (decay[h].T * scale).astype(np.float32)              # [j, i]
            for jh in range(4):
                mk[:, (i * 4 + jh) * B2:(i * 4 + jh + 1) * B2] = (
                    mTh[jh * 128:(jh + 1) * 128, :]
                )
                kdc[:, i * 4 + jh] = kd[h, jh * 128:(jh + 1) * 128]
            qdt[:, i * B2:(i + 1) * B2] = np.broadcast_to(
                qd[h][None, :], (128, B2)
            )
            bdc[:, i] = bd[h]

        in_maps.append(
            {
                "hTb": hTb,
                "w_all": w_allr,
                "wdd": wdr,
                "cosT": cosT,
                "sinT": sinT,
                "maskT": mk,
                "qdtab": qdt.astype(NPBF16),
                "kdcol": kdc,
                "bdcol": bdc,
            }
        )

    global _LAST_IN_MAPS
    _LAST_IN_MAPS = in_maps
    results = bass_utils.run_bass_kernel_spmd(nc, in_maps, core_ids=list(range(M)))

    y_sum = np.zeros((T, HID), np.float64)
    ssq_tot = np.zeros((T,), np.float64)
    for r in range(M):
        y_sum += results.results[r]["y_nat"].astype(np.float64)
        ssq_tot += results.results[r]["ssq"][0].astype(np.float64)
    var = ssq_tot / (H * D)
    F = 1.0 / np.sqrt(var + EPS)
    y = y_sum * F[:, None]
    return y.astype(np.float32)
